# revision 8
# baseline (speedup 1.0000x reference)
"""MLA (DeepSeek-style multi-head latent attention) distributed Bass kernel
for 8 TRN2 NeuronCores.

Problem shapes (hardcoded):
  x (2, 2048, 2048), DIM=2048, N_HEADS=16, Q_LORA=1536, KV_LORA=512,
  QK_NOPE=128, QK_ROPE=64, V_HEAD=128, causal SDPA, scale=192**-0.5.

Distribution:
  phase 1 (token-parallel, 512 tokens/core): q_lora = rmsnorm(x@wq_a.T),
    kv = x@wkv_a.T with rmsnorm on the kv part and rope on the shared k_pe.
  AllGather the 2112-feature activation (feature-major), kv part first.
  phase 2 (head-parallel, 2 heads/core): per-head q/k/v projections + rope(q),
    causal flash attention in the S^T formulation (kt on partitions, exp
    without max-subtraction - scores are provably small for this problem).
  AllToAll attention outputs back to token-parallel (one A2A per local head
    for compute/comm overlap).
  phase 3 (token-parallel): out = attn@wo.T, each core emits its 512 tokens.

All TensorE-facing tensors are float32r (TF32-like matmul, 4x fp32 rate).
Activations are feature-major [features(partitions), tokens(free)] so every
matmul consumes operands natively (no transposes anywhere). RMSNorm weights
are folded into the following projection on the host; rope pair-swap is done
by de-interleaving weight columns on the host (k_pe) or a 128x128 permutation
matmul (q_pe), with the rotation signs folded into a [-s;s;-s;s] sin table.
"""
import sys

sys.path.insert(0, "/opt/trn_rl_repo")

import numpy as np

import concourse.bacc as bacc
import concourse.mybir as mybir
import concourse.tile as tile
from concourse.bass_utils import run_bass_kernel_spmd

F32 = mybir.dt.float32
F32R = mybir.dt.float32r
AFT = mybir.ActivationFunctionType

DIM = 2048
H = 16
QL = 1536
KVL = 512
DN = 128          # qk_nope
DR = 64           # qk_rope
DV = 128          # v head dim
B, S = 2, 2048
T = B * S         # 4096 tokens
NC = 8            # cores
TPC = T // NC     # 512 tokens per core
HPC = H // NC     # 2 heads per core
SCALE = (DN + DR) ** -0.5
EPS = 1e-6
P = 128
CHUNK = 512       # token chunk (= TPC)
NCH = T // CHUNK  # 8 global chunks
KVE = KVL + 2 * DR   # 640: kv_lora + deinterleaved rope + swapped rope
KVG = KVL + DR       # 576 gathered kv feature rows


def build_nc():
    nc = bacc.Bacc("TRN2", target_bir_lowering=False, debug=False, num_devices=NC)

    xT = nc.declare_dram_parameter("xT", [DIM, TPC], F32R, isOutput=False)
    wqaT = nc.declare_dram_parameter("wqaT", [QL // P, P, DIM], F32R, isOutput=False)
    wkvaT = nc.declare_dram_parameter("wkvaT", [KVE // P, P, DIM], F32R, isOutput=False)
    wqbT = nc.declare_dram_parameter("wqbT", [3, P, QL], F32R, isOutput=False)
    wkvbTk = nc.declare_dram_parameter("wkvbTk", [2, P, KVL], F32R, isOutput=False)
    wkvbTv = nc.declare_dram_parameter("wkvbTv", [KVL, 2 * DV], F32R, isOutput=False)
    woT = nc.declare_dram_parameter("woT", [DIM // P, P, DIM], F32R, isOutput=False)
    cosb = nc.declare_dram_parameter("cosb", [P, S], F32R, isOutput=False)
    sinb = nc.declare_dram_parameter("sinb", [P, S], F32R, isOutput=False)
    cosc = nc.declare_dram_parameter("cosc", [P, TPC], F32R, isOutput=False)
    sinc = nc.declare_dram_parameter("sinc", [P, TPC], F32R, isOutput=False)
    perm = nc.declare_dram_parameter("perm", [P, P], F32R, isOutput=False)
    outT = nc.declare_dram_parameter("outT", [DIM, TPC], F32, isOutput=True)

    # internal DRAM for collectives
    agkv_in = nc.dram_tensor("agkv_in", [KVG, TPC], F32R)
    agkv_out = nc.dram_tensor("agkv_out", [NC * KVG, TPC], F32R, addr_space="Shared")
    agq_in = nc.dram_tensor("agq_in", [QL, TPC], F32R)
    agq_out = nc.dram_tensor("agq_out", [NC * QL, TPC], F32R, addr_space="Shared")
    a2a_in = [nc.dram_tensor(f"a2a_in{h}", [NC * DV, CHUNK], F32R) for h in range(HPC)]
    a2a_out = [nc.dram_tensor(f"a2a_out{h}", [NC * DV, CHUNK], F32R)
               for h in range(HPC)]
    groups = [list(range(NC))]

    with tile.TileContext(nc) as tc, \
         tc.tile_pool(name="const", bufs=1) as constp:
        ones_f = constp.tile([P, 1], F32, tag="onesf")
        nc.any.memset(ones_f[:], 1.0)
        ones_col = constp.tile([P, 1], F32R, tag="onesc")
        nc.vector.tensor_copy(ones_col[:], ones_f[:])
        onesr_f = constp.tile([1, P], F32, tag="onesrf")
        nc.any.memset(onesr_f[:], 1.0)
        ones_row = constp.tile([1, P], F32R, tag="onesr")
        nc.vector.tensor_copy(ones_row[:], onesr_f[:])
        eps_t = constp.tile([1, 1], F32, tag="eps")
        nc.any.memset(eps_t[:], EPS)

        # ---------------- phase 1: token-parallel low-rank projections ------
        with tc.tile_pool(name="p1x", bufs=1) as xp, \
             tc.tile_pool(name="p1w", bufs=3) as wp1, \
             tc.tile_pool(name="p1ps", bufs=3, space="PSUM") as ps1, \
             tc.tile_pool(name="p1ssps", bufs=2, space="PSUM") as ssps1, \
             tc.tile_pool(name="p1qa", bufs=1) as qap, \
             tc.tile_pool(name="p1sq", bufs=1) as sqp, \
             tc.tile_pool(name="p1misc", bufs=2) as mp1, \
             tc.tile_pool(name="p1out", bufs=3) as op1:

            cosc_sb = mp1.tile([P, TPC], F32R, tag="cosc")
            nc.sync.dma_start(cosc_sb[:], cosc[:, :])
            sinc_sb = mp1.tile([P, TPC], F32R, tag="sinc")
            nc.sync.dma_start(sinc_sb[:], sinc[:, :])

            xts = []
            for kb in range(DIM // P):
                xt = xp.tile([P, TPC], F32R, tag=f"x{kb}")
                nc.sync.dma_start(xt[:], xT[kb * P:(kb + 1) * P, :])
                xts.append(xt)

            def lora_proj(w_param, nm, out_cb):
                """out m-tile psum = sum_kb w[m][:,kb] @ x[kb]; call out_cb(m, ps)."""
                for m in range(nm):
                    wt = wp1.tile([P, DIM], F32R, tag="w1")
                    nc.sync.dma_start(wt[:], w_param[m, :, :])
                    ps = ps1.tile([P, TPC], F32, tag="ps1")
                    for kb in range(DIM // P):
                        nc.tensor.matmul(ps[:], wt[:, kb * P:(kb + 1) * P], xts[kb][:],
                                         start=(kb == 0), stop=(kb == DIM // P - 1))
                    out_cb(m, ps)

            def rmsnorm_scale(sq_tiles, nfeat):
                """sum-of-squares across partition tiles -> [128,T] inv-rms bcast."""
                ss = ssps1.tile([1, TPC], F32, tag="ss")
                nm = len(sq_tiles)
                for m in range(nm):
                    nc.tensor.matmul(ss[:], ones_col[:], sq_tiles[m][:],
                                     start=(m == 0), stop=(m == nm - 1))
                srt = mp1.tile([1, TPC], F32, tag="srt")
                nc.scalar.activation(srt[:], ss[:], AFT.Sqrt,
                                     bias=eps_t[:], scale=1.0 / nfeat)
                inv = mp1.tile([1, TPC], F32R, tag="inv")
                with nc.allow_low_precision(reason="float32r is fp32 storage"):
                    nc.vector.reciprocal(inv[:], srt[:])
                bc = ps1.tile([P, TPC], F32, tag="ps1")
                nc.tensor.matmul(bc[:], ones_row[:], inv[:], start=True, stop=True)
                binv = mp1.tile([P, TPC], F32R, tag="binv")
                nc.scalar.copy(binv[:], bc[:])
                return binv

            # --- kv branch first (smaller; lets AG_kv fly during q matmuls)
            kva_tiles, sq_kv = [], []

            def kv_cb(m, ps):
                if m < 4:
                    kv = qap.tile([P, TPC], F32R, tag=f"kv{m}")
                    nc.scalar.copy(kv[:], ps[:])
                    kva_tiles.append(kv)
                    sq = sqp.tile([P, TPC], F32R, tag=f"skv{m}")
                    nc.scalar.square(sq[:], ps[:])
                    sq_kv.append(sq)
                else:
                    # rows 0:64 = deinterleaved kpe [r;i], rows 64:128 = [i;r]
                    t_a = op1.tile([DR, TPC], F32R, tag="ropea")
                    nc.vector.tensor_mul(t_a[:], ps[0:DR, :], cosc_sb[0:DR, :])
                    t_b = op1.tile([DR, TPC], F32R, tag="ropeb")
                    nc.vector.tensor_mul(t_b[:], ps[DR:2 * DR, :], sinc_sb[0:DR, :])
                    kpe = op1.tile([DR, TPC], F32R, tag="ropeo")
                    nc.vector.tensor_add(kpe[:], t_a[:], t_b[:])
                    nc.sync.dma_start(agkv_in[KVL:KVL + DR, :], kpe[:])

            lora_proj(wkvaT, KVE // P, kv_cb)
            binv_kv = rmsnorm_scale(sq_kv, KVL)
            for m in range(4):
                kvn = op1.tile([P, TPC], F32R, tag="normout")
                nc.vector.tensor_mul(kvn[:], kva_tiles[m][:], binv_kv[:])
                nc.sync.dma_start(agkv_in[m * P:(m + 1) * P, :], kvn[:])

            nc.gpsimd.collective_compute(
                "AllGather", mybir.AluOpType.bypass, replica_groups=groups,
                ins=[agkv_in.ap().opt()], outs=[agkv_out.ap().opt()])

            # --- q branch
            qa_tiles, sq_q = [], []

            def q_cb(m, ps):
                qa = qap.tile([P, TPC], F32R, tag=f"qa{m}")
                nc.scalar.copy(qa[:], ps[:])
                qa_tiles.append(qa)
                sq = sqp.tile([P, TPC], F32R, tag=f"sq{m}")
                nc.scalar.square(sq[:], ps[:])
                sq_q.append(sq)

            lora_proj(wqaT, QL // P, q_cb)
            binv_q = rmsnorm_scale(sq_q, QL)
            for m in range(QL // P):
                qn = op1.tile([P, TPC], F32R, tag="normout")
                nc.vector.tensor_mul(qn[:], qa_tiles[m][:], binv_q[:])
                nc.sync.dma_start(agq_in[m * P:(m + 1) * P, :], qn[:])

            nc.gpsimd.collective_compute(
                "AllGather", mybir.AluOpType.bypass, replica_groups=groups,
                ins=[agq_in.ap().opt()], outs=[agq_out.ap().opt()])

        # ---------------- phase 2: head-parallel q/k/v + attention ----------
        with tc.tile_pool(name="p2q", bufs=1) as qp2, \
             tc.tile_pool(name="p2k", bufs=1) as kp2, \
             tc.tile_pool(name="p2v", bufs=T // P) as vp2:

            q_n = [qp2.tile([P, T], F32R, tag=f"qn{h}", name=f"qn{h}") for h in range(HPC)]
            q_rope = qp2.tile([P, T], F32R, tag="qrope")
            k_n = [kp2.tile([P, T], F32R, tag=f"kn{h}", name=f"kn{h}") for h in range(HPC)]
            # k_pe duplicated into both partition halves so that each head's
            # rope matmul has lhsT/rhs at the same base partition (h*64)
            k_pe = kp2.tile([2 * DR, T], F32R, tag="kpe")
            v_tok = [vp2.tile([P, HPC * DV], F32R, tag="vtok", name=f"vtok{i}") for i in range(T // P)]

            with tc.tile_pool(name="p2cs", bufs=2) as csp, \
                 tc.tile_pool(name="p2w", bufs=1) as wp2, \
                 tc.tile_pool(name="p2actq", bufs=3) as actp, \
                 tc.tile_pool(name="p2actkv", bufs=5) as actkvp, \
                 tc.tile_pool(name="p2misc", bufs=1) as mp2, \
                 tc.tile_pool(name="p2psq", bufs=1, space="PSUM") as psq, \
                 tc.tile_pool(name="p2psk", bufs=1, space="PSUM") as psk, \
                 tc.tile_pool(name="p2psv", bufs=2, space="PSUM") as psv:
                perm_sb = mp2.tile([P, P], F32R, tag="perm")
                nc.sync.dma_start(perm_sb[:], perm[:, :])
                wqb_sb = []
                for m in range(3):
                    w = wp2.tile([P, QL], F32R, tag=f"wqb{m}", name=f"wqb{m}")
                    nc.sync.dma_start(w[:], wqbT[m, :, :])
                    wqb_sb.append(w)
                wkvk_sb = []
                for m in range(2):
                    w = wp2.tile([P, KVL], F32R, tag=f"wkvk{m}", name=f"wkvk{m}")
                    nc.sync.dma_start(w[:], wkvbTk[m, :, :])
                    wkvk_sb.append(w)
                wkvv_sb = []
                for kb in range(KVL // P):
                    w = wp2.tile([P, 2 * DV], F32R, tag=f"wkvv{kb}", name=f"wkvv{kb}")
                    nc.sync.dma_start(w[:], wkvbTv[kb * P:(kb + 1) * P, :])
                    wkvv_sb.append(w)
                for qc in range(NCH):
                    c0 = qc * CHUNK
                    pss = [psq.tile([P, CHUNK], F32, tag=f"qps{m}", name=f"qps{m}") for m in range(3)]
                    for kb in range(QL // P):
                        at = actp.tile([P, CHUNK], F32R, tag="actq")
                        nc.sync.dma_start(
                            at[:], agq_out[qc * QL + kb * P: qc * QL + (kb + 1) * P, :])
                        for m in range(3):
                            nc.tensor.matmul(pss[m][:], wqb_sb[m][:, kb * P:(kb + 1) * P],
                                             at[:],
                                             start=(kb == 0), stop=(kb == QL // P - 1))
                    for h in range(HPC):
                        nc.scalar.copy(q_n[h][:, c0:c0 + CHUNK], pss[h][:])
                    # q rope: evict raw, swap via perm matmul, combine with cos/sin
                    qr_raw = mp2.tile([P, CHUNK], F32R, tag="qrraw")
                    nc.scalar.copy(qr_raw[:], pss[2][:])
                    ps_sw = psq.tile([P, CHUNK], F32, tag="qps0")
                    nc.tensor.matmul(ps_sw[:], perm_sb[:], qr_raw[:], start=True, stop=True)
                    pcol = c0 % S
                    cos_q = csp.tile([P, CHUNK], F32R, tag="cosq")
                    nc.sync.dma_start(cos_q[:], cosb[:, pcol:pcol + CHUNK])
                    sin_q = csp.tile([P, CHUNK], F32R, tag="sinq")
                    nc.sync.dma_start(sin_q[:], sinb[:, pcol:pcol + CHUNK])
                    t_a = mp2.tile([P, CHUNK], F32R, tag="qra")
                    nc.vector.tensor_mul(t_a[:], qr_raw[:], cos_q[:])
                    t_b = mp2.tile([P, CHUNK], F32R, tag="qrb")
                    nc.vector.tensor_mul(t_b[:], ps_sw[:], sin_q[:])
                    nc.vector.tensor_add(q_rope[:, c0:c0 + CHUNK], t_a[:], t_b[:])

                    # k/v from kv activations
                    actkv_ts = []
                    for kb in range(KVL // P):
                        at = actkvp.tile([P, CHUNK], F32R, tag="actkv")
                        nc.sync.dma_start(
                            at[:], agkv_out[qc * KVG + kb * P: qc * KVG + (kb + 1) * P, :])
                        actkv_ts.append(at)
                    kps = [psk.tile([P, CHUNK], F32, tag=f"kps{m}", name=f"kps{m}") for m in range(2)]
                    for kb in range(KVL // P):
                        for m in range(2):
                            nc.tensor.matmul(kps[m][:], wkvk_sb[m][:, kb * P:(kb + 1) * P],
                                             actkv_ts[kb][:],
                                             start=(kb == 0), stop=(kb == KVL // P - 1))
                    for h in range(HPC):
                        nc.scalar.copy(k_n[h][:, c0:c0 + CHUNK], kps[h][:])
                    nc.sync.dma_start(
                        k_pe[0:DR, c0:c0 + CHUNK],
                        agkv_out[qc * KVG + KVL: qc * KVG + KVL + DR, :])
                    nc.sync.dma_start(
                        k_pe[DR:2 * DR, c0:c0 + CHUNK],
                        agkv_out[qc * KVG + KVL: qc * KVG + KVL + DR, :])
                    for ktc in range(CHUNK // P):
                        vps = psv.tile([P, HPC * DV], F32, tag="vps")
                        for kb in range(KVL // P):
                            nc.tensor.matmul(vps[:],
                                             actkv_ts[kb][:, ktc * P:(ktc + 1) * P],
                                             wkvv_sb[kb][:],
                                             start=(kb == 0), stop=(kb == KVL // P - 1))
                        nc.scalar.copy(v_tok[qc * (CHUNK // P) + ktc][:], vps[:])

            # ----- causal attention, S^T formulation; head-outer for A2A overlap
            with tc.tile_pool(name="apt", bufs=3) as ptp, \
                 tc.tile_pool(name="aout", bufs=2, space="PSUM") as outp, \
                 tc.tile_pool(name="aden", bufs=2, space="PSUM") as denp, \
                 tc.tile_pool(name="ast", bufs=2, space="PSUM") as stp, \
                 tc.tile_pool(name="abc", bufs=1, space="PSUM") as bcp, \
                 tc.tile_pool(name="afin", bufs=2) as finp:
                for h in range(HPC):
                    for b in range(B):
                        for qcl in range(S // CHUNK):
                            qg = b * (S // CHUNK) + qcl
                            q0 = qg * CHUNK
                            nkt = (CHUNK // P) * (qcl + 1)
                            out_ps = outp.tile([P, CHUNK], F32, tag="out")
                            den_ps = denp.tile([1, CHUNK], F32, tag="den")
                            for kt in range(nkt):
                                kcol = b * S + kt * P
                                st_ps = stp.tile([P, CHUNK], F32, tag="st")
                                nc.tensor.matmul(st_ps[:],
                                                 k_n[h][:, kcol:kcol + P],
                                                 q_n[h][:, q0:q0 + CHUNK],
                                                 start=True, stop=False)
                                nc.tensor.matmul(st_ps[:],
                                                 k_pe[h * DR:(h + 1) * DR, kcol:kcol + P],
                                                 q_rope[h * DR:(h + 1) * DR, q0:q0 + CHUNK],
                                                 start=False, stop=True)
                                pt = ptp.tile([P, CHUNK], F32R, tag="pt")
                                nc.scalar.activation(pt[:], st_ps[:], AFT.Exp,
                                                     bias=0.0, scale=SCALE)
                                if kt >= (CHUNK // P) * qcl:  # diagonal block
                                    ptm = ptp.tile([P, CHUNK], F32R, tag="ptm")
                                    nc.gpsimd.affine_select(
                                        ptm[:], pt[:], pattern=[[1, CHUNK]],
                                        base=qcl * CHUNK - kt * P,
                                        channel_multiplier=-1,
                                        compare_op=mybir.AluOpType.is_ge, fill=0.0)
                                    ptf = ptm
                                else:
                                    ptf = pt
                                nc.tensor.matmul(
                                    out_ps[:],
                                    v_tok[(b * S // P) + kt][:, h * DV:(h + 1) * DV],
                                    ptf[:],
                                    start=(kt == 0), stop=(kt == nkt - 1))
                                nc.tensor.matmul(den_ps[:], ones_col[:], ptf[:],
                                                 start=(kt == 0), stop=(kt == nkt - 1))
                            dinv = finp.tile([1, CHUNK], F32R, tag="dinv")
                            with nc.allow_low_precision(reason="float32r is fp32 storage"):
                                nc.vector.reciprocal(dinv[:], den_ps[:])
                            bc_ps = bcp.tile([P, CHUNK], F32, tag="bc")
                            nc.tensor.matmul(bc_ps[:], ones_row[:], dinv[:],
                                             start=True, stop=True)
                            binv_a = finp.tile([P, CHUNK], F32R, tag="binva")
                            nc.scalar.copy(binv_a[:], bc_ps[:])
                            attn = finp.tile([P, CHUNK], F32R, tag="attn")
                            nc.vector.tensor_mul(attn[:], out_ps[:], binv_a[:])
                            nc.sync.dma_start(a2a_in[h][qg * DV:(qg + 1) * DV, :],
                                              attn[:])
                    nc.gpsimd.collective_compute(
                        "AllToAll", mybir.AluOpType.bypass, replica_groups=groups,
                        ins=[a2a_in[h].ap().opt()], outs=[a2a_out[h].ap().opt()])

        # ---------------- phase 3: token-parallel output projection ---------
        with tc.tile_pool(name="p3r", bufs=1) as rp3, \
             tc.tile_pool(name="p3w", bufs=3) as wp3, \
             tc.tile_pool(name="p3ps", bufs=2, space="PSUM") as ps3, \
             tc.tile_pool(name="p3o", bufs=3) as op3:
            rhs_t = []
            for g in range(H):  # global head g lives in a2a_out[g % HPC], block g//HPC
                rt = rp3.tile([P, CHUNK], F32R, tag=f"r{g}")
                nc.sync.dma_start(rt[:], a2a_out[g % HPC][(g // HPC) * DV:
                                                          (g // HPC + 1) * DV, :])
                rhs_t.append(rt)
            # contract first-of-pair heads first so phase 3 overlaps second A2A
            kt_order = [g for g in range(H) if g % 2 == 0] + \
                       [g for g in range(H) if g % 2 == 1]
            for m in range(DIM // P):
                wt = wp3.tile([P, DIM], F32R, tag="wo")
                nc.sync.dma_start(wt[:], woT[m, :, :])
                ps = ps3.tile([P, CHUNK], F32, tag="ps3")
                for i, g in enumerate(kt_order):
                    nc.tensor.matmul(ps[:], wt[:, g * P:(g + 1) * P], rhs_t[g][:],
                                     start=(i == 0), stop=(i == H - 1))
                ot = op3.tile([P, CHUNK], F32, tag="ot")
                nc.scalar.copy(ot[:], ps[:])
                nc.sync.dma_start(outT[m * P:(m + 1) * P, :], ot[:])

    nc.compile()
    return nc


def _tile_kxm(w, nk, nm):
    """(nk*128, nm*128) -> (nm, 128, nk*128): [m][p][kt*128+j] = w[kt*128+p, m*128+j]."""
    return np.ascontiguousarray(
        w.reshape(nk, P, nm, P).transpose(2, 1, 0, 3).reshape(nm, P, nk * P))


_CACHE = {}


def _prep(inputs):
    x = np.asarray(inputs["x"], np.float32)
    fc = np.asarray(inputs["freqs_cos"], np.float32)
    fs = np.asarray(inputs["freqs_sin"], np.float32)
    wq_a = np.asarray(inputs["wq_a"], np.float32)
    q_norm_w = np.asarray(inputs["q_norm_w"], np.float32)
    wq_b = np.asarray(inputs["wq_b"], np.float32)
    wkv_a = np.asarray(inputs["wkv_a"], np.float32)
    kv_norm_w = np.asarray(inputs["kv_norm_w"], np.float32)
    wkv_b = np.asarray(inputs["wkv_b"], np.float32)
    wo = np.asarray(inputs["wo"], np.float32)

    x_flat = x.reshape(T, DIM)

    wqaT_t = _tile_kxm(wq_a.T, DIM // P, QL // P)

    at = wkv_a.T                                     # (DIM, 576)
    Rw = at[:, KVL::2]                               # 32 real rope cols
    Iw = at[:, KVL + 1::2]
    wkvaT_t = _tile_kxm(np.concatenate([at[:, :KVL], Rw, Iw, Iw, Rw], axis=1),
                        DIM // P, KVE // P)

    wqb_sT = (wq_b * q_norm_w[None, :]).T            # (QL, H*192)
    wkvb_sT = (wkv_b * kv_norm_w[None, :]).T         # (KVL, H*256)

    woT_t = _tile_kxm(wo.T, DIM // P, DIM // P)

    cT, sT = fc.T, fs.T                              # (32, S)
    cosb = np.concatenate([cT, cT, cT, cT], axis=0)  # (128, S)
    sinb = np.concatenate([-sT, sT, -sT, sT], axis=0)
    permM = np.zeros((P, P), np.float32)
    permM[np.arange(P) ^ 32, np.arange(P)] = 1.0

    in_maps = []
    for c in range(NC):
        h0, h1 = 2 * c, 2 * c + 1
        qb = [wqb_sT[:, h * 192: h * 192 + DN] for h in (h0, h1)]
        for h in (h0, h1):
            rope = wqb_sT[:, h * 192 + DN:(h + 1) * 192]
            qb.append(rope[:, 0::2])
            qb.append(rope[:, 1::2])
        wqbT_ct = _tile_kxm(np.concatenate(qb, axis=1), QL // P, 3)

        kn = [wkvb_sT[:, h * 256: h * 256 + DN] for h in (h0, h1)]
        vv = [wkvb_sT[:, h * 256 + DN: (h + 1) * 256] for h in (h0, h1)]
        wkvbTk_c = _tile_kxm(np.concatenate(kn, axis=1), KVL // P, 2)
        wkvbTv_c = np.ascontiguousarray(np.concatenate(vv, axis=1))  # (KVL, 256)

        pos0 = (c * TPC) % S
        in_maps.append({
            "xT": np.ascontiguousarray(x_flat[c * TPC:(c + 1) * TPC].T),
            "wqaT": wqaT_t, "wkvaT": wkvaT_t,
            "wqbT": wqbT_ct, "wkvbTk": wkvbTk_c, "wkvbTv": wkvbTv_c,
            "woT": woT_t, "cosb": cosb, "sinb": sinb, "perm": permM,
            "cosc": np.ascontiguousarray(cosb[:, pos0:pos0 + TPC]),
            "sinc": np.ascontiguousarray(sinb[:, pos0:pos0 + TPC]),
        })
    return in_maps


def kernel(**inputs):
    in_maps = _prep(inputs)
    if "nc" not in _CACHE:
        _CACHE["nc"] = build_nc()
    r = run_bass_kernel_spmd(_CACHE["nc"], in_maps, list(range(NC)))
    out_flat = np.empty((T, DIM), np.float32)
    for c in range(NC):
        out_flat[c * TPC:(c + 1) * TPC] = r.results[c]["outT"].T
    return out_flat.reshape(B, S, DIM)


# revision 15
# speedup vs baseline: 1.4215x; 1.4215x over previous
"""MLA (DeepSeek-style multi-head latent attention) distributed Bass kernel
for 8 TRN2 NeuronCores.

Problem shapes (hardcoded):
  x (2, 2048, 2048), DIM=2048, N_HEADS=16, Q_LORA=1536, KV_LORA=512,
  QK_NOPE=128, QK_ROPE=64, V_HEAD=128, causal SDPA, scale=192**-0.5.

Distribution / overlap strategy:
  phase 1 (token-parallel, 512 tokens/core): q_lora = x@wq_a.T and
    kv = x@wkv_a.T, shipped UNNORMALIZED together with per-token inv-rms rows
    so each AllGather can launch as soon as its slice of matmuls finishes
    (RMSNorm would otherwise couple all features and serialize compute->AG).
    Rope is applied to the shared k_pe here (per-token). Three AllGathers:
    kv (577 rows), q first half (768 rows), q second half (769 rows).
  phase 2 (head-parallel, 2 heads/core): k/v production (gated only on AG_kv)
    runs while the q AllGathers are in flight; then q production; then causal
    flash attention in the S^T formulation (kt on partitions, exp without
    max-subtraction - scores are provably small here). RMSNorm is applied at
    production time: row-broadcast multiplies for k/q (token axis = free) and
    per-partition activation scales for v (token axis = partitions).
  AllToAll per local head ships UNNORMALIZED attention outputs + softmax
    denominators (129-row shards); normalization happens in phase 3, keeping
    the attention inner loop free of serializing reductions.
  phase 3 (token-parallel): normalize per head, then out = attn@wo.T.

All matmul operands are bfloat16 (fp32 PSUM accumulation); fp32 is used for
the rms/softmax statistics chains. Activations are feature-major
[features(partitions), tokens(free)] so every matmul consumes operands
natively - there are no transposes anywhere in the kernel.
"""
import sys

sys.path.insert(0, "/opt/trn_rl_repo")

import numpy as np
import ml_dtypes

import concourse.bacc as bacc
import concourse.mybir as mybir
import concourse.tile as tile
from concourse.bass_utils import run_bass_kernel_spmd

BF = ml_dtypes.bfloat16
F32 = mybir.dt.float32
CD = mybir.dt.bfloat16
AFT = mybir.ActivationFunctionType

DIM = 2048
H = 16
QL = 1536
KVL = 512
DN = 128          # qk_nope
DR = 64           # qk_rope
DV = 128          # v head dim
B, S = 2, 2048
T = B * S
NC = 8
TPC = T // NC     # 512 tokens per core
HPC = H // NC     # 2 heads per core
SCALE = (DN + DR) ** -0.5
EPS = 1e-6
P = 128
CHUNK = 512
NCH = T // CHUNK
KVE = KVL + 2 * DR   # 640 phase-1 kv output cols (incl swapped-rope block)
KVG = KVL + DR + 1   # 577 gathered kv rows (+ inv_rms row)
QG1 = QL // 2        # 768
QG2 = QL // 2 + 1    # 769 (+ inv_rms row)


def build_nc():
    nc = bacc.Bacc("TRN2", target_bir_lowering=False, debug=False, num_devices=NC)

    xT = nc.declare_dram_parameter("xT", [DIM, TPC], CD, isOutput=False)
    wqaT = nc.declare_dram_parameter("wqaT", [QL // P, P, DIM], CD, isOutput=False)
    wkvaT = nc.declare_dram_parameter("wkvaT", [KVE // P, P, DIM], CD, isOutput=False)
    wqbT = nc.declare_dram_parameter("wqbT", [3, P, QL], CD, isOutput=False)
    wkvbTk = nc.declare_dram_parameter("wkvbTk", [2, P, KVL], CD, isOutput=False)
    wkvbTv = nc.declare_dram_parameter("wkvbTv", [KVL, 2 * DV], CD, isOutput=False)
    woT = nc.declare_dram_parameter("woT", [DIM // P, P, DIM], CD, isOutput=False)
    cosb = nc.declare_dram_parameter("cosb", [P, S], CD, isOutput=False)
    sinb = nc.declare_dram_parameter("sinb", [P, S], CD, isOutput=False)
    cosc = nc.declare_dram_parameter("cosc", [P, TPC], CD, isOutput=False)
    sinc = nc.declare_dram_parameter("sinc", [P, TPC], CD, isOutput=False)
    perm = nc.declare_dram_parameter("perm", [P, P], CD, isOutput=False)
    outT = nc.declare_dram_parameter("outT", [DIM, TPC], F32, isOutput=True)

    agkv_in = nc.dram_tensor("agkv_in", [KVG, TPC], CD)
    agkv_out = nc.dram_tensor("agkv_out", [NC * KVG, TPC], CD, addr_space="Shared")
    agq1_in = nc.dram_tensor("agq1_in", [QG1, TPC], CD)
    agq1_out = nc.dram_tensor("agq1_out", [NC * QG1, TPC], CD, addr_space="Shared")
    agq2_in = nc.dram_tensor("agq2_in", [QG2, TPC], CD)
    agq2_out = nc.dram_tensor("agq2_out", [NC * QG2, TPC], CD, addr_space="Shared")
    a2a_in = [nc.dram_tensor(f"a2a_in{h}", [NC * (DV + 1), CHUNK], CD)
              for h in range(HPC)]
    a2a_out = [nc.dram_tensor(f"a2a_out{h}", [NC * (DV + 1), CHUNK], CD)
               for h in range(HPC)]
    groups = [list(range(NC))]

    with tile.TileContext(nc) as tc, \
         tc.tile_pool(name="const", bufs=1) as constp:
        ones_f = constp.tile([P, 1], F32, tag="onesf")
        nc.any.memset(ones_f[:], 1.0)
        ones_col = constp.tile([P, 1], CD, tag="onesc")
        nc.vector.tensor_copy(ones_col[:], ones_f[:])
        onesr_f = constp.tile([1, P], F32, tag="onesrf")
        nc.any.memset(onesr_f[:], 1.0)
        ones_row = constp.tile([1, P], CD, tag="onesr")
        nc.vector.tensor_copy(ones_row[:], onesr_f[:])
        one_f11 = constp.tile([1, 1], F32, tag="onef11")
        nc.any.memset(one_f11[:], 1.0)
        one_1x1 = constp.tile([1, 1], CD, tag="one11")
        nc.vector.tensor_copy(one_1x1[:], one_f11[:])
        eps_t = constp.tile([1, 1], F32, tag="eps")
        nc.any.memset(eps_t[:], EPS)

        # persistent phase-2 tensors; zero-fills go first on the gpsimd queue
        _qp2cm = tc.tile_pool(name="p2q", bufs=1)
        qp2 = _qp2cm.__enter__()
        _kp2cm = tc.tile_pool(name="p2k", bufs=1)
        kp2 = _kp2cm.__enter__()
        _vp2cm = tc.tile_pool(name="p2v", bufs=T // P)
        vp2 = _vp2cm.__enter__()
        q_n = [qp2.tile([P, T], CD, tag=f"qn{h}", name=f"qn{h}") for h in range(HPC)]
        q_rope = [qp2.tile([P, T], CD, tag=f"qrope{h}", name=f"qrope{h}")
                  for h in range(HPC)]
        k_n = [kp2.tile([P, T], CD, tag=f"kn{h}", name=f"kn{h}") for h in range(HPC)]
        k_pe = kp2.tile([2 * DR, T], CD, tag="kpe")
        v_tok = [vp2.tile([P, HPC * DV], CD, tag="vtok", name=f"vtok{i}")
                 for i in range(T // P)]
        nc.any.memset(k_pe[DR:2 * DR, :], 0.0)
        for h in range(HPC):
            nc.any.memset(q_rope[h][DR:2 * DR, :], 0.0)

        # ---------------- phase 1 ------------------------------------------
        with tc.tile_pool(name="p1x", bufs=1) as xp, \
             tc.tile_pool(name="p1w", bufs=3) as wp1, \
             tc.tile_pool(name="p1ps", bufs=3, space="PSUM") as ps1, \
             tc.tile_pool(name="p1ssps", bufs=2, space="PSUM") as ssps1, \
             tc.tile_pool(name="p1sq", bufs=1) as sqp, \
             tc.tile_pool(name="p1misc", bufs=2) as mp1, \
             tc.tile_pool(name="p1out", bufs=4) as op1:

            xts = []
            for kb in range(DIM // P):
                xt = xp.tile([P, TPC], CD, tag=f"x{kb}")
                nc.sync.dma_start(xt[:], xT[kb * P:(kb + 1) * P, :])
                xts.append(xt)

            cosc_sb = mp1.tile([P, TPC], CD, tag="cosc")
            nc.sync.dma_start(cosc_sb[:], cosc[:, :])
            sinc_sb = mp1.tile([P, TPC], CD, tag="sinc")
            nc.sync.dma_start(sinc_sb[:], sinc[:, :])

            def lora_proj(w_param, nm, out_cb):
                for m in range(nm):
                    wt = wp1.tile([P, DIM], CD, tag="w1")
                    nc.sync.dma_start(wt[:], w_param[m, :, :])
                    ps = ps1.tile([P, TPC], F32, tag="ps1")
                    for kb in range(DIM // P):
                        nc.tensor.matmul(ps[:], wt[:, kb * P:(kb + 1) * P], xts[kb][:],
                                         start=(kb == 0), stop=(kb == DIM // P - 1))
                    out_cb(m, ps)

            def invrms_row(sq_tiles, nfeat, dst, dst_row):
                """sumsq -> 1/rms row (CD) -> DMA to dst[dst_row]."""
                ss = ssps1.tile([1, TPC], F32, tag="ss")
                nm = len(sq_tiles)
                for m in range(nm):
                    nc.tensor.matmul(ss[:], ones_col[:], sq_tiles[m][:],
                                     start=(m == 0), stop=(m == nm - 1))
                srt = mp1.tile([1, TPC], F32, tag="srt")
                nc.scalar.activation(srt[:], ss[:], AFT.Sqrt,
                                     bias=eps_t[:], scale=1.0 / nfeat)
                inv_f = mp1.tile([1, TPC], F32, tag="invf")
                nc.vector.reciprocal_approx_fast(inv_f[:], srt[:])
                inv = mp1.tile([1, TPC], CD, tag="inv")
                nc.vector.tensor_copy(inv[:], inv_f[:])
                nc.sync.dma_start(dst[dst_row:dst_row + 1, :], inv[:])

            # --- kv branch (payload shipped unnormalized + inv-rms row)
            sq_kv = []

            def kv_cb(m, ps):
                if m < 4:
                    kv = op1.tile([P, TPC], CD, tag="pay")
                    nc.scalar.copy(kv[:], ps[:])
                    nc.sync.dma_start(agkv_in[m * P:(m + 1) * P, :], kv[:])
                    sq = sqp.tile([P, TPC], CD, tag=f"skv{m}", name=f"skv{m}")
                    nc.scalar.square(sq[:], ps[:])
                    sq_kv.append(sq)
                else:
                    # rows 0:64 deinterleaved kpe [r;i], rows 64:128 [i;r]
                    t_a = op1.tile([DR, TPC], CD, tag="ropea")
                    nc.vector.tensor_mul(t_a[:], ps[0:DR, :], cosc_sb[0:DR, :])
                    t_b = op1.tile([DR, TPC], CD, tag="ropeb")
                    nc.vector.tensor_mul(t_b[:], ps[DR:2 * DR, :], sinc_sb[0:DR, :])
                    kpe = op1.tile([DR, TPC], CD, tag="ropeo")
                    nc.vector.tensor_add(kpe[:], t_a[:], t_b[:])
                    nc.sync.dma_start(agkv_in[KVL:KVL + DR, :], kpe[:])

            lora_proj(wkvaT, KVE // P, kv_cb)
            invrms_row(sq_kv, KVL, agkv_in, KVL + DR)

            nc.gpsimd.collective_compute(
                "AllGather", mybir.AluOpType.bypass, replica_groups=groups,
                ins=[agkv_in.ap().opt()], outs=[agkv_out.ap().opt()])

            # --- q branch, split across two AllGathers
            sq_q = []

            def q_cb(m, ps):
                qa = op1.tile([P, TPC], CD, tag="pay")
                nc.scalar.copy(qa[:], ps[:])
                if m < 6:
                    nc.sync.dma_start(agq1_in[m * P:(m + 1) * P, :], qa[:])
                else:
                    nc.sync.dma_start(agq2_in[(m - 6) * P:(m - 5) * P, :], qa[:])
                sq = sqp.tile([P, TPC], CD, tag=f"sq{m}", name=f"sq{m}")
                nc.scalar.square(sq[:], ps[:])
                sq_q.append(sq)
                if m == 5:
                    nc.gpsimd.collective_compute(
                        "AllGather", mybir.AluOpType.bypass, replica_groups=groups,
                        ins=[agq1_in.ap().opt()], outs=[agq1_out.ap().opt()])

            lora_proj(wqaT, QL // P, q_cb)
            invrms_row(sq_q, QL, agq2_in, QL // 2)

            nc.gpsimd.collective_compute(
                "AllGather", mybir.AluOpType.bypass, replica_groups=groups,
                ins=[agq2_in.ap().opt()], outs=[agq2_out.ap().opt()])

        # ---------------- phase 2 ------------------------------------------
        if True:
            with tc.tile_pool(name="p2cs", bufs=2) as csp, \
                 tc.tile_pool(name="p2w", bufs=1) as wp2, \
                 tc.tile_pool(name="p2actq", bufs=4) as actp, \
                 tc.tile_pool(name="p2actkv", bufs=5) as actkvp, \
                 tc.tile_pool(name="p2misc", bufs=2) as mp2, \
                 tc.tile_pool(name="p2nrm", bufs=2) as nrmp:
                perm_sb = mp2.tile([P, P], CD, tag="perm")
                nc.sync.dma_start(perm_sb[:], perm[:, :])
                wqb_sb = []
                for m in range(3):
                    w = wp2.tile([P, QL], CD, tag=f"wqb{m}", name=f"wqb{m}")
                    nc.sync.dma_start(w[:], wqbT[m, :, :])
                    wqb_sb.append(w)
                wkvk_sb = []
                for m in range(2):
                    w = wp2.tile([P, KVL], CD, tag=f"wkvk{m}", name=f"wkvk{m}")
                    nc.sync.dma_start(w[:], wkvbTk[m, :, :])
                    wkvk_sb.append(w)
                wkvv_sb = []
                for kb in range(KVL // P):
                    w = wp2.tile([P, 2 * DV], CD, tag=f"wkvv{kb}", name=f"wkvv{kb}")
                    nc.sync.dma_start(w[:], wkvbTv[kb * P:(kb + 1) * P, :])
                    wkvv_sb.append(w)

                # --- k/v production (gated only on AG_kv)
                kvscope = tc.tile_pool(name="p2psk", bufs=1, space="PSUM")
                psk = kvscope.__enter__()
                vscope = tc.tile_pool(name="p2psv", bufs=3, space="PSUM")
                psv = vscope.__enter__()
                bscope = tc.tile_pool(name="p2psb", bufs=1, space="PSUM")
                psb = bscope.__enter__()
                for qc in range(NCH):
                    c0 = qc * CHUNK
                    actkv_ts = []
                    for kb in range(KVL // P):
                        at = actkvp.tile([P, CHUNK], CD, tag="actkv")
                        nc.sync.dma_start(
                            at[:],
                            agkv_out[qc * KVG + kb * P: qc * KVG + (kb + 1) * P, :])
                        actkv_ts.append(at)
                    ikv = nrmp.tile([1, CHUNK], CD, tag="ikv")
                    nc.sync.dma_start(
                        ikv[:],
                        agkv_out[qc * KVG + KVL + DR: qc * KVG + KVL + DR + 1, :])
                    bkv_ps = psb.tile([P, CHUNK], F32, tag="bkv")
                    nc.tensor.matmul(bkv_ps[:], ones_row[:], ikv[:],
                                     start=True, stop=True)
                    bkv = nrmp.tile([P, CHUNK], CD, tag="bkvsb")
                    nc.scalar.copy(bkv[:], bkv_ps[:])
                    kps = [psk.tile([P, CHUNK], F32, tag=f"kps{m}", name=f"kps{m}")
                           for m in range(2)]
                    for kb in range(KVL // P):
                        for m in range(2):
                            nc.tensor.matmul(kps[m][:],
                                             wkvk_sb[m][:, kb * P:(kb + 1) * P],
                                             actkv_ts[kb][:],
                                             start=(kb == 0), stop=(kb == KVL // P - 1))
                    for h in range(HPC):
                        nc.vector.tensor_mul(k_n[h][:, c0:c0 + CHUNK], kps[h][:], bkv[:])
                    nc.sync.dma_start(
                        k_pe[0:DR, c0:c0 + CHUNK],
                        agkv_out[qc * KVG + KVL: qc * KVG + KVL + DR, :])
                    for ktc in range(CHUNK // P):
                        # inv_kv as a [128,1] column for this kt tile (K=1 matmul)
                        col_ps = psb.tile([P, 1], F32, tag="colp")
                        nc.tensor.matmul(col_ps[:],
                                         ikv[0:1, ktc * P:(ktc + 1) * P],
                                         one_1x1[:], start=True, stop=True)
                        vcol = nrmp.tile([P, 1], F32, tag="vcol")
                        nc.scalar.copy(vcol[:], col_ps[:])
                        vps = psv.tile([P, HPC * DV], F32, tag="vps")
                        for kb in range(KVL // P):
                            nc.tensor.matmul(vps[:],
                                             actkv_ts[kb][:, ktc * P:(ktc + 1) * P],
                                             wkvv_sb[kb][:],
                                             start=(kb == 0), stop=(kb == KVL // P - 1))
                        nc.scalar.activation(v_tok[qc * (CHUNK // P) + ktc][:], vps[:],
                                             AFT.Copy, bias=0.0, scale=vcol[:])
                bscope.__exit__(None, None, None)
                vscope.__exit__(None, None, None)
                kvscope.__exit__(None, None, None)

                # --- q production (kb 0..5 from AG_q1, 6..11 from AG_q2)
                qscope = tc.tile_pool(name="p2psq", bufs=2, space="PSUM")
                psq = qscope.__enter__()
                qbscope = tc.tile_pool(name="p2psqb", bufs=2, space="PSUM")
                psqb = qbscope.__enter__()
                for qc in range(NCH):
                    c0 = qc * CHUNK
                    iq = nrmp.tile([1, CHUNK], CD, tag="iq")
                    nc.sync.dma_start(
                        iq[:],
                        agq2_out[qc * QG2 + QL // 2: qc * QG2 + QL // 2 + 1, :])
                    bq_ps = psqb.tile([P, CHUNK], F32, tag="bq")
                    nc.tensor.matmul(bq_ps[:], ones_row[:], iq[:], start=True, stop=True)
                    bq = nrmp.tile([P, CHUNK], CD, tag="bqsb")
                    nc.scalar.copy(bq[:], bq_ps[:])
                    pss = [psq.tile([P, CHUNK], F32, tag=f"qps{m}", name=f"qps{m}")
                           for m in range(3)]
                    for kb in range(QL // P):
                        at = actp.tile([P, CHUNK], CD, tag="actq")
                        if kb < 6:
                            nc.sync.dma_start(
                                at[:],
                                agq1_out[qc * QG1 + kb * P: qc * QG1 + (kb + 1) * P, :])
                        else:
                            nc.sync.dma_start(
                                at[:],
                                agq2_out[qc * QG2 + (kb - 6) * P:
                                         qc * QG2 + (kb - 5) * P, :])
                        for m in range(3):
                            nc.tensor.matmul(pss[m][:],
                                             wqb_sb[m][:, kb * P:(kb + 1) * P],
                                             at[:],
                                             start=(kb == 0), stop=(kb == QL // P - 1))
                    for h in range(HPC):
                        nc.vector.tensor_mul(q_n[h][:, c0:c0 + CHUNK], pss[h][:], bq[:])
                    qr_raw = mp2.tile([P, CHUNK], CD, tag="qrraw")
                    nc.vector.tensor_mul(qr_raw[:], pss[2][:], bq[:])
                    ps_sw = psq.tile([P, CHUNK], F32, tag="qps0")
                    nc.tensor.matmul(ps_sw[:], perm_sb[:], qr_raw[:],
                                     start=True, stop=True)
                    pcol = c0 % S
                    cos_q = csp.tile([P, CHUNK], CD, tag="cosq")
                    nc.sync.dma_start(cos_q[:], cosb[:, pcol:pcol + CHUNK])
                    sin_q = csp.tile([P, CHUNK], CD, tag="sinq")
                    nc.sync.dma_start(sin_q[:], sinb[:, pcol:pcol + CHUNK])
                    t_a = mp2.tile([P, CHUNK], CD, tag="qra")
                    nc.vector.tensor_mul(t_a[:], qr_raw[:], cos_q[:])
                    t_b = mp2.tile([P, CHUNK], CD, tag="qrb")
                    nc.vector.tensor_mul(t_b[:], ps_sw[:], sin_q[:])
                    qr_fin = mp2.tile([P, CHUNK], CD, tag="qrfin")
                    nc.vector.tensor_add(qr_fin[:], t_a[:], t_b[:])
                    for h in range(HPC):
                        nc.sync.dma_start(q_rope[h][0:DR, c0:c0 + CHUNK],
                                          qr_fin[h * DR:(h + 1) * DR, :])
                qbscope.__exit__(None, None, None)
                qscope.__exit__(None, None, None)

            # ----- causal attention (S^T formulation), head-outer for A2A overlap
            with tc.tile_pool(name="apt", bufs=3) as ptp, \
                 tc.tile_pool(name="aout", bufs=2, space="PSUM") as outp, \
                 tc.tile_pool(name="aden", bufs=2, space="PSUM") as denp, \
                 tc.tile_pool(name="ast", bufs=3, space="PSUM") as stp, \
                 tc.tile_pool(name="afin", bufs=3) as finp:
                for h in range(HPC):
                    for b in range(B):
                        for qcl in range(S // CHUNK):
                            qg = b * (S // CHUNK) + qcl
                            q0 = qg * CHUNK
                            nkt = (CHUNK // P) * (qcl + 1)
                            out_ps = outp.tile([P, CHUNK], F32, tag="out")
                            den_ps = denp.tile([1, CHUNK], F32, tag="den")
                            for kt in range(nkt):
                                kcol = b * S + kt * P
                                st_ps = stp.tile([P, CHUNK], F32, tag="st")
                                nc.tensor.matmul(st_ps[:],
                                                 k_n[h][:, kcol:kcol + P],
                                                 q_n[h][:, q0:q0 + CHUNK],
                                                 start=True, stop=False)
                                nc.tensor.matmul(st_ps[:],
                                                 k_pe[:, kcol:kcol + P],
                                                 q_rope[h][:, q0:q0 + CHUNK],
                                                 start=False, stop=True)
                                pt = ptp.tile([P, CHUNK], CD, tag="pt")
                                nc.scalar.activation(pt[:], st_ps[:], AFT.Exp,
                                                     bias=0.0, scale=SCALE)
                                if kt >= (CHUNK // P) * qcl:  # diagonal block
                                    ptm = ptp.tile([P, CHUNK], CD, tag="ptm")
                                    nc.gpsimd.affine_select(
                                        ptm[:], pt[:], pattern=[[1, CHUNK]],
                                        base=qcl * CHUNK - kt * P,
                                        channel_multiplier=-1,
                                        compare_op=mybir.AluOpType.is_ge, fill=0.0)
                                    ptf = ptm
                                else:
                                    ptf = pt
                                nc.tensor.matmul(
                                    out_ps[:],
                                    v_tok[(b * S // P) + kt][:, h * DV:(h + 1) * DV],
                                    ptf[:],
                                    start=(kt == 0), stop=(kt == nkt - 1))
                                nc.tensor.matmul(den_ps[:], ones_col[:], ptf[:],
                                                 start=(kt == 0), stop=(kt == nkt - 1))
                            attn = finp.tile([P, CHUNK], CD, tag="attn")
                            nc.vector.tensor_copy(attn[:], out_ps[:])
                            den_c = finp.tile([1, CHUNK], CD, tag="denc")
                            nc.scalar.copy(den_c[:], den_ps[:])
                            nc.sync.dma_start(
                                a2a_in[h][qg * (DV + 1):qg * (DV + 1) + DV, :], attn[:])
                            nc.sync.dma_start(
                                a2a_in[h][qg * (DV + 1) + DV:(qg + 1) * (DV + 1), :],
                                den_c[:])
                    nc.gpsimd.collective_compute(
                        "AllToAll", mybir.AluOpType.bypass, replica_groups=groups,
                        ins=[a2a_in[h].ap().opt()], outs=[a2a_out[h].ap().opt()])

        _vp2cm.__exit__(None, None, None)
        _kp2cm.__exit__(None, None, None)
        _qp2cm.__exit__(None, None, None)

        # ---------------- phase 3 ------------------------------------------
        with tc.tile_pool(name="p3r", bufs=1) as rp3, \
             tc.tile_pool(name="p3w", bufs=1) as wp3, \
             tc.tile_pool(name="p3ps", bufs=2, space="PSUM") as ps3, \
             tc.tile_pool(name="p3bc", bufs=2, space="PSUM") as bcp, \
             tc.tile_pool(name="p3o", bufs=3) as op3:
            # heads arriving via the first A2A load+normalize first; the woT
            # weight loads sit between the two gated groups so they prefetch
            # before the second A2A lands
            rhs_t = [None] * H
            order = [g for g in range(H) if g % HPC == 0]
            order2 = [g for g in range(H) if g % HPC != 0]

            def load_head(g):
                blk = (g // HPC) * (DV + 1)
                rt = rp3.tile([P, CHUNK], CD, tag=f"r{g}", name=f"r{g}")
                nc.sync.dma_start(rt[:], a2a_out[g % HPC][blk:blk + DV, :])
                dr = rp3.tile([1, CHUNK], CD, tag=f"d{g}", name=f"d{g}")
                nc.sync.dma_start(dr[:], a2a_out[g % HPC][blk + DV:blk + DV + 1, :])
                bc_ps = bcp.tile([P, CHUNK], F32, tag="bc")
                nc.tensor.matmul(bc_ps[:], ones_row[:], dr[:], start=True, stop=True)
                binv = rp3.tile([P, CHUNK], F32, tag=f"bi{g}", name=f"bi{g}")
                nc.vector.reciprocal_approx_fast(binv[:], bc_ps[:])
                rn = rp3.tile([P, CHUNK], CD, tag=f"rn{g}", name=f"rn{g}")
                nc.vector.tensor_mul(rn[:], rt[:], binv[:])
                rhs_t[g] = rn

            for g in order:
                load_head(g)
            wo_sb = []
            for m in range(DIM // P):
                wt = wp3.tile([P, DIM], CD, tag=f"wo{m}", name=f"wo{m}")
                nc.sync.dma_start(wt[:], woT[m, :, :])
                wo_sb.append(wt)
            for g in order2:
                load_head(g)

            kt_order = order + order2
            for m in range(DIM // P):
                ps = ps3.tile([P, CHUNK], F32, tag="ps3")
                for i, g in enumerate(kt_order):
                    nc.tensor.matmul(ps[:], wo_sb[m][:, g * P:(g + 1) * P], rhs_t[g][:],
                                     start=(i == 0), stop=(i == H - 1))
                ot = op3.tile([P, CHUNK], F32, tag="ot")
                nc.scalar.copy(ot[:], ps[:])
                nc.sync.dma_start(outT[m * P:(m + 1) * P, :], ot[:])

    nc.compile()
    return nc


def _tile_kxm(w, nk, nm):
    """(nk*128, nm*128) -> (nm, 128, nk*128): [m][p][kt*128+j] = w[kt*128+p, m*128+j]."""
    return np.ascontiguousarray(
        w.reshape(nk, P, nm, P).transpose(2, 1, 0, 3).reshape(nm, P, nk * P))


_CACHE = {}


def _prep(inputs):
    x = np.asarray(inputs["x"], np.float32)
    fc = np.asarray(inputs["freqs_cos"], np.float32)
    fs = np.asarray(inputs["freqs_sin"], np.float32)
    wq_a = np.asarray(inputs["wq_a"], np.float32)
    q_norm_w = np.asarray(inputs["q_norm_w"], np.float32)
    wq_b = np.asarray(inputs["wq_b"], np.float32)
    wkv_a = np.asarray(inputs["wkv_a"], np.float32)
    kv_norm_w = np.asarray(inputs["kv_norm_w"], np.float32)
    wkv_b = np.asarray(inputs["wkv_b"], np.float32)
    wo = np.asarray(inputs["wo"], np.float32)

    x_flat = x.reshape(T, DIM)

    wqaT_t = _tile_kxm(wq_a.T, DIM // P, QL // P)

    at = wkv_a.T                                     # (DIM, 576)
    Rw = at[:, KVL::2]
    Iw = at[:, KVL + 1::2]
    wkvaT_t = _tile_kxm(np.concatenate([at[:, :KVL], Rw, Iw, Iw, Rw], axis=1),
                        DIM // P, KVE // P)

    wqb_sT = (wq_b * q_norm_w[None, :]).T            # (QL, H*192)
    wkvb_sT = (wkv_b * kv_norm_w[None, :]).T         # (KVL, H*256)

    woT_t = _tile_kxm(wo.T, DIM // P, DIM // P)

    cT, sT = fc.T, fs.T
    cosbM = np.concatenate([cT, cT, cT, cT], axis=0)
    sinbM = np.concatenate([-sT, sT, -sT, sT], axis=0)
    permM = np.zeros((P, P), np.float32)
    permM[np.arange(P) ^ 32, np.arange(P)] = 1.0

    in_maps = []
    for c in range(NC):
        h0, h1 = 2 * c, 2 * c + 1
        qb = [wqb_sT[:, h * 192: h * 192 + DN] for h in (h0, h1)]
        for h in (h0, h1):
            rope = wqb_sT[:, h * 192 + DN:(h + 1) * 192]
            qb.append(rope[:, 0::2])
            qb.append(rope[:, 1::2])
        wqbT_ct = _tile_kxm(np.concatenate(qb, axis=1), QL // P, 3)

        kn = [wkvb_sT[:, h * 256: h * 256 + DN] for h in (h0, h1)]
        vv = [wkvb_sT[:, h * 256 + DN: (h + 1) * 256] for h in (h0, h1)]
        wkvbTk_c = _tile_kxm(np.concatenate(kn, axis=1), KVL // P, 2)
        wkvbTv_c = np.ascontiguousarray(np.concatenate(vv, axis=1))

        pos0 = (c * TPC) % S
        in_maps.append({
            "xT": np.ascontiguousarray(x_flat[c * TPC:(c + 1) * TPC].T).astype(BF),
            "wqaT": wqaT_t.astype(BF), "wkvaT": wkvaT_t.astype(BF),
            "wqbT": wqbT_ct.astype(BF), "wkvbTk": wkvbTk_c.astype(BF),
            "wkvbTv": wkvbTv_c.astype(BF),
            "woT": woT_t.astype(BF), "cosb": cosbM.astype(BF),
            "sinb": sinbM.astype(BF), "perm": permM.astype(BF),
            "cosc": np.ascontiguousarray(cosbM[:, pos0:pos0 + TPC]).astype(BF),
            "sinc": np.ascontiguousarray(sinbM[:, pos0:pos0 + TPC]).astype(BF),
        })
    return in_maps


def kernel(**inputs):
    in_maps = _prep(inputs)
    if "nc" not in _CACHE:
        _CACHE["nc"] = build_nc()
    r = run_bass_kernel_spmd(_CACHE["nc"], in_maps, list(range(NC)))
    out_flat = np.empty((T, DIM), np.float32)
    for c in range(NC):
        out_flat[c * TPC:(c + 1) * TPC] = r.results[c]["outT"].T
    return out_flat.reshape(B, S, DIM)
